# revision 16
# baseline (speedup 1.0000x reference)
"""BiLSTM-CRF Trainium2 kernel (8 NeuronCores, axon).

Pipeline (4 SPMD launches, host does only sharding/reshaping between them):
  L1 (8 cores, seq-sharded): embedding gather + input projections x@W_ih.T+b
  L2 (core0 fwd / core1 bwd): the sequential LSTM recurrence, 4096 steps
  L3 (core0 fwd / core1 bwd): hidden2tag features + Viterbi max-plus scans
  host: pointwise fv+bv argmax decode (elementwise) + path score

Self-contained: only needs numpy/ml_dtypes/concourse (from /opt/trn_rl_repo).
"""
import os
import sys
import time

for _p in ("/opt/trn_rl_repo", "/root/.axon_site/_ro/trn_rl_repo"):
    if os.path.isdir(_p) and _p not in sys.path:
        sys.path.insert(0, _p)

import numpy as np
import ml_dtypes

import concourse.bass as bass
import concourse.mybir as mybir
import concourse.tile as tile
from concourse.bass import ds, IndirectOffsetOnAxis
from concourse.bass_utils import run_bass_kernel_spmd
from concourse.masks import make_identity

F32 = mybir.dt.float32
BF16 = mybir.dt.bfloat16
I32 = mybir.dt.int32
AF = mybir.ActivationFunctionType
OP = mybir.AluOpType
BF16NP = ml_dtypes.bfloat16

# model constants (fixed by the problem)
VSIZE, EMB, HID, TAGS = 50000, 512, 1024, 8
H = HID // 2          # 512 per direction
G4 = 4 * H            # 2048 gate dim
START, STOP = 6, 7
NEG = -10000.0
SEQ_FULL = 4096
N_CORES = 8

# gate permutation: torch order i,f,g,o -> ours i,f,o,g (sigmoid block contiguous)
_PERM = np.concatenate([
    np.arange(0, H),            # i
    np.arange(H, 2 * H),        # f
    np.arange(3 * H, 4 * H),    # o
    np.arange(2 * H, 3 * H),    # g~
])

VERBOSE = bool(int(os.environ.get("KERNEL_VERBOSE", "0")))
TRACE = bool(int(os.environ.get("KERNEL_TRACE", "0")))

_ws_ctr = [0]


def _split_multi_waits(nc):
    """walrus in this env allows 1 sync wait per instruction; hoist extras to NoOps."""
    for fn in nc.m.functions:
        for bb in fn.blocks:
            out, changed = [], False
            for inst in bb.instructions:
                si = inst.sync_info
                waits = list(si.on_wait) if (si is not None and si.on_wait) else []
                if len(waits) > 1:
                    changed = True
                    for w in waits[:-1]:
                        _ws_ctr[0] += 1
                        out.append(mybir.InstNoOp(
                            name=f"WS-{_ws_ctr[0]}", engine=inst.engine, ins=[],
                            outs=[],
                            sync_info=mybir.SyncInfo(on_wait=[w], on_update=[]),
                        ))
                    inst.sync_info = mybir.SyncInfo(
                        on_wait=[waits[-1]], on_update=list(si.on_update or []))
                out.append(inst)
            if changed:
                bb.instructions = out
    return nc


# ----------------------------------------------------------------------------
# L1: embedding gather + x @ W_ih.T + b for one sequence chunk (both dirs)
# ----------------------------------------------------------------------------
def build_l1(chunk, repeat=1):
    nt = chunk // 128
    nc = bass.Bass()
    sent = nc.dram_tensor("sent", [chunk], I32, kind="ExternalInput")
    wemb = nc.dram_tensor("wemb", [VSIZE, EMB], F32, kind="ExternalInput")
    # W_ih.T (perm'd): [EMB, G4] f32, both directions
    wf = nc.dram_tensor("wf", [EMB, G4], F32, kind="ExternalInput")
    wb = nc.dram_tensor("wb", [EMB, G4], F32, kind="ExternalInput")
    bf_in = nc.dram_tensor("bf", [128, 16], F32, kind="ExternalInput")
    bb_in = nc.dram_tensor("bb", [128, 16], F32, kind="ExternalInput")
    # outputs, layout B: col = j*chunk + t  (j = gate row-block 0..15)
    xpf = nc.dram_tensor("xpf", [128, 16 * chunk], F32, kind="ExternalOutput")
    xpb = nc.dram_tensor("xpb", [128, 16 * chunk], F32, kind="ExternalOutput")

    with tile.TileContext(nc) as tc:
        with tc.tile_pool(name="const", bufs=1) as cp, \
             tc.tile_pool(name="work", bufs=2) as wp, \
             tc.tile_pool(name="ps", bufs=4, space="PSUM") as pp:
            ident = cp.tile([128, 128], F32, tag="ident")
            make_identity(nc, ident[:])
            w_sb = {}
            for name, dram in (("f", wf), ("b", wb)):
                t = cp.tile([128, 4 * G4], F32, tag=f"w{name}", name=f"w{name}")
                # [p, ec*G4 + r] = W.T[ec*128+p, r]
                nc.sync.dma_start(t[:].rearrange("p (ec r) -> p ec r", ec=4),
                                  dram.rearrange("(ec p) r -> p ec r", p=128))
                w_sb[name] = t
            b_sb = {}
            for name, dram in (("f", bf_in), ("b", bb_in)):
                t = cp.tile([128, 16], F32, tag=f"b{name}", name=f"bsb{name}")
                nc.sync.dma_start(t[:], dram[:])
                b_sb[name] = t

            rep_ctx = tc.For_i(0, repeat, 1) if repeat > 1 else None
            if rep_ctx is not None:
                rep_ctx.__enter__()
            for ti in range(nt):
                t0 = ti * 128
                idx = wp.tile([128, 1], I32, tag="idx")
                nc.sync.dma_start(idx[:], sent[t0:t0 + 128].rearrange("(p o) -> p o", o=1))
                emb = wp.tile([128, EMB], F32, tag="emb")
                nc.gpsimd.indirect_dma_start(
                    out=emb[:], out_offset=None, in_=wemb[:],
                    in_offset=IndirectOffsetOnAxis(ap=idx[:, 0:1], axis=0))
                # transpose emb -> embT [e, t] in 4 chunks
                embT = wp.tile([128, 4 * 128], F32, tag="embT")
                for e in range(4):
                    pt = pp.tile([128, 128], F32, tag="ptr")
                    nc.tensor.transpose(pt[:], emb[:, e * 128:(e + 1) * 128], ident[:])
                    nc.scalar.copy(embT[:, e * 128:(e + 1) * 128], pt[:])
                # x-projections, both dirs: psum[j] [128r, 128t]
                for name, out_dram in (("f", xpf), ("b", xpb)):
                    for j in range(16):
                        ps = pp.tile([128, 128], F32, tag="pmm")
                        for e in range(4):
                            nc.tensor.matmul(
                                ps[:],
                                w_sb[name][:, e * G4 + j * 128: e * G4 + (j + 1) * 128],
                                embT[:, e * 128:(e + 1) * 128],
                                start=(e == 0), stop=(e == 3))
                        ob = wp.tile([128, 128], F32, tag="ob")
                        nc.vector.tensor_scalar_add(ob[:], ps[:], b_sb[name][:, j:j + 1])
                        nc.sync.dma_start(out_dram[:, j * chunk + t0: j * chunk + t0 + 128], ob[:])
            if rep_ctx is not None:
                rep_ctx.__exit__(None, None, None)
    return _split_multi_waits(nc)


# ----------------------------------------------------------------------------
# L2: the LSTM recurrence (one direction per core, data-driven)
# ----------------------------------------------------------------------------
def build_l2(seq, t_blk=32, frac=1, no_mv=False, no_gates=False, no_dma=False, mv_j=16, repeat=1):
    n_pairs = seq // (2 * t_blk) // frac
    nc = bass.Bass()
    # W_hh as 64 lhsT blocks: w[p, (j*4+k)*128 + m] = W_hh.T[k*128+p, j*128+m]
    w_in = nc.dram_tensor("whh", [128, 64 * 128], F32, kind="ExternalInput")
    # x-proj(+bias): [p, j, t] with t padded by 2*t_blk
    xp_in = nc.dram_tensor("xp", [128, 16, seq + 2 * t_blk], F32, kind="ExternalInput")
    hT_out = nc.dram_tensor("hT", [128, 4 * seq], F32, kind="ExternalOutput")

    with tile.TileContext(nc) as tc:
        with tc.tile_pool(name="const", bufs=1) as cp, \
             tc.tile_pool(name="gates", bufs=3) as gp, \
             tc.tile_pool(name="ps", bufs=4, space="PSUM") as pp:
            w_sb = cp.tile([128, 64 * 128], F32, tag="w")
            nc.sync.dma_start(w_sb[:], w_in[:])
            c_state = cp.tile([128, 4], F32, tag="c")
            nc.vector.memset(c_state[:], 0.0)
            # block-diagonal h: col 5*k holds h chunk k, rest zeros (rhs N=4 trick)
            h4 = cp.tile([128, 16], F32, tag="h4")
            nc.vector.memset(h4[:], 0.0)
            xp_sb = [cp.tile([128, 16 * t_blk], F32, tag=f"xp{i}", name=f"xp{i}") for i in range(2)]
            h_sb = [cp.tile([128, 4 * t_blk], F32, tag=f"h{i}", name=f"hsb{i}") for i in range(2)]
            # h_{-1} = 0 lives in the tail slot of h_sb[1]
            nc.vector.memset(h_sb[1][:], 0.0)

            def load_xp(buf, blk_iv):
                src = xp_in[:, :, ds(blk_iv * t_blk, t_blk)]
                nc.sync.dma_start(xp_sb[buf][:].rearrange("p (j t) -> p j t", t=t_blk), src)

            def store_h(buf, blk_iv):
                dst = hT_out.rearrange("p (c t) -> p c t", c=4)[
                    :, :, ds(blk_iv * t_blk, t_blk)]
                src = h_sb[buf][:].rearrange("p (c t) -> p c t", t=t_blk)
                nc.sync.dma_start(dst, src)

            def step(xbuf, hbuf, s, h_prev):
                h_view = h_sb[hbuf][:].rearrange("p (c t) -> p c t", t=t_blk)
                ps = pp.tile([128, 64], F32, tag="mv")
                if not no_mv:
                    for j in range(mv_j):
                        for k in range(4):
                            blk = (j * 4 + k) * 128
                            nc.tensor.matmul(
                                ps[:, j * 4:(j + 1) * 4], w_sb[:, blk:blk + 128],
                                h4[:, k * 4:(k + 1) * 4],
                                start=(k == 0), stop=(k == 3))
                if no_gates:
                    if no_mv:
                        return (hbuf, s)
                    nc.vector.tensor_scalar_add(h_view[:, :, s], ps[:, 0:4], 1.0)
                    return (hbuf, s)
                gr = gp.tile([128, 16], F32, tag="gr")
                nc.vector.tensor_reduce(
                    gr[:], ps[:].rearrange("p (j c) -> p j c", c=4),
                    axis=mybir.AxisListType.X, op=OP.add)
                g = gp.tile([128, 16], F32, tag="g")
                xps = xp_sb[xbuf][:].rearrange("p (j t) -> p j t", t=t_blk)[:, :, s]
                if no_mv:
                    nc.vector.tensor_copy(g[:], xps)
                else:
                    nc.vector.tensor_tensor(g[:], gr[:], xps, op=OP.add)
                sig = gp.tile([128, 12], F32, tag="sig")
                nc.scalar.activation(sig[:], g[:, 0:12], AF.Sigmoid)
                tg = gp.tile([128, 4], F32, tag="tg")
                nc.scalar.activation(tg[:], g[:, 12:16], AF.Tanh)
                t1 = gp.tile([128, 4], F32, tag="t1")
                nc.vector.tensor_tensor(t1[:], sig[:, 0:4], tg[:], op=OP.mult)
                t2 = gp.tile([128, 4], F32, tag="t2")
                nc.vector.tensor_tensor(t2[:], sig[:, 4:8], c_state[:], op=OP.mult)
                nc.vector.tensor_tensor(c_state[:], t1[:], t2[:], op=OP.add)
                tc2 = gp.tile([128, 4], F32, tag="tc2")
                nc.scalar.activation(tc2[:], c_state[:], AF.Tanh)
                nc.vector.tensor_tensor(h_view[:, :, s], sig[:, 8:12], tc2[:],
                                        op=OP.mult)
                nc.vector.tensor_copy(h4[:, 0:16:5], h_view[:, :, s])
                return (hbuf, s)

            load_xp(0, 0)  # prologue
            rep_ctx = tc.For_i(0, repeat, 1) if repeat > 1 else None
            if rep_ctx is not None:
                rep_ctx.__enter__()
            with tc.For_i(0, n_pairs, 1) as iv:
                if not no_dma:
                    load_xp(1, 2 * iv + 1)
                h_prev = None
                for s in range(t_blk):
                    h_prev = step(0, 0, s, h_prev)
                if not no_dma:
                    store_h(0, 2 * iv)
                    load_xp(0, 2 * iv + 2)  # overruns into pad on last iter
                for s in range(t_blk):
                    h_prev = step(1, 1, s, h_prev)
                if not no_dma:
                    store_h(1, 2 * iv + 1)
            if rep_ctx is not None:
                rep_ctx.__exit__(None, None, None)
    return _split_multi_waits(nc)


# ----------------------------------------------------------------------------
# L3: hidden2tag features + Viterbi max-plus scan (direction via data)
# ----------------------------------------------------------------------------
def build_l3(seq, frac=1, repeat=1):
    nt = seq // 128
    nc = bass.Bass()
    hf_in = nc.dram_tensor("hf", [128, 4 * seq], F32, kind="ExternalInput")
    hb_in = nc.dram_tensor("hb", [128, 4 * seq], F32, kind="ExternalInput")
    # w_tag.T chunks: [p, c*8 + j] = w_tag.T[c*128+p, j], c = 0..7
    wt_in = nc.dram_tensor("wt", [128, 64], F32, kind="ExternalInput")
    btag_in = nc.dram_tensor("btag", [8, 1], F32, kind="ExternalInput")
    m_in = nc.dram_tensor("m", [8, 8], F32, kind="ExternalInput")       # T or T.T
    init_in = nc.dram_tensor("init", [8, 1], F32, kind="ExternalInput")
    maskpre_in = nc.dram_tensor("mpre", [8, 1], F32, kind="ExternalInput")
    maskpost_in = nc.dram_tensor("mpost", [8, 1], F32, kind="ExternalInput")
    vh_out = nc.dram_tensor("vh", [8, seq + 1], F32, kind="ExternalOutput")
    ft_out = nc.dram_tensor("ft", [8, seq], F32, kind="ExternalOutput")

    with tile.TileContext(nc) as tc:
        with tc.tile_pool(name="const", bufs=1) as cp, \
             tc.tile_pool(name="ps", bufs=4, space="PSUM") as pp:
            h_sb = {}
            for name, dram in (("f", hf_in), ("b", hb_in)):
                t = cp.tile([128, 4 * seq], F32, tag=f"h{name}", name=f"hin{name}")
                nc.sync.dma_start(t[:], dram[:])
                h_sb[name] = t
            wt = cp.tile([128, 64], F32, tag="wt")
            nc.sync.dma_start(wt[:], wt_in[:])
            btag = cp.tile([8, 1], F32, tag="btag")
            nc.sync.dma_start(btag[:], btag_in[:])
            m_sb = cp.tile([8, 8], F32, tag="m")
            nc.sync.dma_start(m_sb[:], m_in[:])
            mpre = cp.tile([8, 1], F32, tag="mpre")
            nc.sync.dma_start(mpre[:], maskpre_in[:])
            mpost = cp.tile([8, 1], F32, tag="mpost")
            nc.sync.dma_start(mpost[:], maskpost_in[:])

            ft = cp.tile([8, seq], F32, tag="ft")
            for ti in range(nt):
                ps = pp.tile([8, 128], F32, tag="ftp")
                for c in range(8):
                    hs = h_sb["f"] if c < 4 else h_sb["b"]
                    cc = c % 4
                    nc.tensor.matmul(
                        ps[:], wt[:, c * 8:(c + 1) * 8],
                        hs[:, cc * seq + ti * 128: cc * seq + (ti + 1) * 128],
                        start=(c == 0), stop=(c == 7))
                nc.vector.tensor_scalar_add(ft[:, ti * 128:(ti + 1) * 128], ps[:], btag[:, 0:1])
            nc.sync.dma_start(ft_out[:], ft[:])

            pre = cp.tile([8, seq], F32, tag="pre")
            nc.vector.tensor_scalar(pre[:], ft[:], mpre[:, 0:1], None, op0=OP.mult)
            post = cp.tile([8, seq], F32, tag="post")
            nc.vector.tensor_scalar(post[:], ft[:], mpost[:, 0:1], None, op0=OP.mult)

            vh = cp.tile([8, seq + 1], F32, tag="vh")
            iv0 = cp.tile([8, 1], F32, tag="iv0")
            nc.sync.dma_start(iv0[:], init_in[:])
            nc.vector.tensor_copy(vh[:, 0:1], iv0[:])

            in32 = cp.tile([32, 32], F32, tag="in32")
            nc.vector.memset(in32[:], 0.0)
            tr32 = cp.tile([32, 32], F32, tag="tr32")
            nc.vector.memset(tr32[:], 0.0)
            a8 = cp.tile([8, 1], F32, tag="a8")
            sc8 = cp.tile([8, 8], F32, tag="sc8")
            mx8 = cp.tile([8, 1], F32, tag="mx8")

            rep_ctx = tc.For_i(0, repeat, 1) if repeat > 1 else None
            if rep_ctx is not None:
                rep_ctx.__enter__()
            with tc.For_i(0, seq // frac, 1) as iv:
                nc.vector.tensor_tensor(a8[:], vh[:, ds(iv, 1)], pre[:, ds(iv, 1)], op=OP.add)
                nc.vector.tensor_copy(in32[0:8, 0:8], a8[:].to_broadcast([8, 8]))
                nc.vector.transpose(tr32[:], in32[:])
                nc.vector.tensor_tensor(sc8[:], tr32[0:8, 0:8], m_sb[:], op=OP.add)
                nc.vector.tensor_reduce(mx8[:], sc8[:], axis=mybir.AxisListType.X, op=OP.max)
                nc.vector.tensor_tensor(vh[:, ds(iv + 1, 1)], mx8[:], post[:, ds(iv, 1)], op=OP.add)
            if rep_ctx is not None:
                rep_ctx.__exit__(None, None, None)
            nc.sync.dma_start(vh_out[:], vh[:])
    return _split_multi_waits(nc)


# ----------------------------------------------------------------------------
# host orchestration
# ----------------------------------------------------------------------------
_cache = {}


def _get(name, builder, *args):
    key = (name,) + args
    if key not in _cache:
        t0 = time.time()
        _cache[key] = builder(*args)
        if VERBOSE:
            print(f"[kernel] built {key} in {time.time()-t0:.1f}s", flush=True)
    return _cache[key]


def _run(nc, in_maps, label):
    if label in ("L1", "L2", "L3"):
        _last_im[label] = in_maps
    t0 = time.time()
    try:
        res = run_bass_kernel_spmd(nc, in_maps, list(range(len(in_maps))),
                                   trace=TRACE)
    except ModuleNotFoundError:
        res = run_bass_kernel_spmd(nc, in_maps, list(range(len(in_maps))),
                                   trace=False)
    wall = time.time() - t0
    if VERBOSE:
        print(f"[kernel] {label}: {wall:.2f}s exec_ns={res.exec_time_ns}",
              flush=True)
    _run.exec_ns[label] = res.exec_time_ns
    _run.wall_s[label] = wall
    return res


_run.exec_ns = {}
_run.wall_s = {}
_last_im = {}


def _prep_l2_w(w_hh):
    """w_hh [G4, H] f32 (perm'd rows) -> [128, 64*128] bf16 lhsT blocks."""
    wt = w_hh.T.astype(np.float32)  # [H, G4] = [k*128+p, j*128+m]
    out = np.empty((128, 64 * 128), dtype=np.float32)
    for j in range(16):
        for k in range(4):
            blk = (j * 4 + k) * 128
            out[:, blk:blk + 128] = wt[k * 128:(k + 1) * 128, j * 128:(j + 1) * 128]
    return out


def kernel(**inputs):
    seq = int(np.asarray(inputs["sentence"]).shape[0])
    sentence = np.asarray(inputs["sentence"]).astype(np.int32)
    wembed = np.asarray(inputs["wembed"], np.float32)
    transitions = np.asarray(inputs["transitions"], np.float32)
    w_tag = np.asarray(inputs["w_tag"], np.float32)
    b_tag = np.asarray(inputs["b_tag"], np.float32)

    wih = {}
    whh = {}
    bias = {}
    for d, suf in (("f", "_f"), ("b", "_b")):
        wih[d] = np.asarray(inputs["w_ih" + suf], np.float32)[_PERM]
        whh[d] = np.asarray(inputs["w_hh" + suf], np.float32)[_PERM]
        bias[d] = np.asarray(inputs["b" + suf], np.float32)[_PERM]

    chunk = seq // N_CORES
    # ---- L1: gather + projections ------------------------------------------
    nc1 = _get("l1", build_l1, chunk)
    in_maps = []
    for c in range(N_CORES):
        in_maps.append(dict(
            sent=sentence[c * chunk:(c + 1) * chunk],
            wemb=wembed,
            wf=np.ascontiguousarray(wih["f"].T),
            wb=np.ascontiguousarray(wih["b"].T),
            bf=np.ascontiguousarray(bias["f"].reshape(16, 128).T),
            bb=np.ascontiguousarray(bias["b"].reshape(16, 128).T),
        ))
    r1 = _run(nc1, in_maps, "L1")
    xp = {}
    for d, nm in (("f", "xpf"), ("b", "xpb")):
        full = np.empty((128, 16, seq), np.float32)
        for c in range(N_CORES):
            full[:, :, c * chunk:(c + 1) * chunk] = \
                r1.results[c][nm].reshape(128, 16, chunk)
        xp[d] = full

    # ---- L2: recurrences ---------------------------------------------------
    w2 = {d: _prep_l2_w(whh[d]) for d in ("f", "b")}
    xp_f_lay = xp["f"].reshape(128, 16 * seq)
    xp_b_rev = xp["b"][:, :, ::-1].reshape(128, 16 * seq)
    nc2 = _get("l2", build_l2, seq)
    im2 = []
    for c in range(N_CORES):
        d = "f" if c == 0 else ("b" if c == 1 else "f")
        x = np.zeros((128, 16, seq + 64), np.float32)
        x[:, :, :seq] = (xp_f_lay if d == "f" else xp_b_rev).reshape(128, 16, seq)
        im2.append(dict(whh=w2[d], xp=x))
    r2 = _run(nc2, im2, "L2")
    hf_T = np.asarray(r2.results[0]["hT"], np.float32).reshape(128, 4, seq)
    hb_T = np.asarray(r2.results[1]["hT"], np.float32).reshape(128, 4, seq)[:, :, ::-1]

    # ---- L3: feats + viterbi scans -----------------------------------------
    wtp = np.zeros((128, 64), np.float32)
    wtt = w_tag.T.astype(np.float32)  # [HID, TAGS]
    for c in range(8):
        wtp[:, c * 8:(c + 1) * 8] = wtt[c * 128:(c + 1) * 128]
    init_f = np.full((8, 1), NEG, np.float32)
    init_f[START] = 0.0
    init_b = transitions[STOP].reshape(8, 1).astype(np.float32)
    ones = np.ones((8, 1), np.float32)
    zeros = np.zeros((8, 1), np.float32)
    btag_col = b_tag.reshape(8, 1).astype(np.float32)

    def h_bytes(a):  # [128, 4, seq] bf16 -> [128, 4*seq] contiguous
        return np.ascontiguousarray(a.reshape(128, 4 * seq))

    im_f = dict(hf=h_bytes(hf_T), hb=h_bytes(hb_T), wt=wtp, btag=btag_col,
                m=transitions.astype(np.float32), init=init_f,
                mpre=zeros, mpost=ones)
    im_b = dict(hf=h_bytes(hf_T[:, :, ::-1]), hb=h_bytes(hb_T[:, :, ::-1]),
                wt=wtp, btag=btag_col,
                m=np.ascontiguousarray(transitions.T), init=init_b,
                mpre=ones, mpost=zeros)
    nc3 = _get("l3", build_l3, seq)
    im3 = [im_f, im_b] + [im_f] * (N_CORES - 2)
    r3 = _run(nc3, im3, "L3")
    fv = r3.results[0]["vh"][:, 1:seq + 1]          # fv_t at col t
    bv_rev = r3.results[1]["vh"][:, 0:seq]          # bv_t at col seq-1-t
    bv = bv_rev[:, ::-1]

    # ---- host decode (elementwise) -----------------------------------------
    total = fv + bv                                  # [8, seq]
    best_path = np.argmax(total, axis=0).astype(np.int32)
    terminal = fv[:, seq - 1] + transitions[STOP]
    path_score = np.float32(np.max(terminal))
    return np.asarray(path_score, np.float32), best_path


if __name__ == "__main__":
    # quick self-run with random data at reduced seq for debugging
    pass
